# revision 14
# baseline (speedup 1.0000x reference)
"""Bass/Tile kernel for a 4-layer dense transformer (prefill) on 8 TRN2 cores.

Parallelization: 8-way sequence parallelism over the 2048 concatenated
tokens (2 batches x 1024). Core c owns global token block c (256 tokens:
batch c//4, positions (c%4)*256..). The residual stream x is token-sharded,
kept TRANSPOSED [feature(partition), token(free)] in fp32.

Per layer:
  - LN1/LN2 fully local (each core has all features for its tokens).
  - AllGather(h1) [0.5MB bf16 in, 8-core mesh] replicates LN1 output.
  - Attention head-sharded: core c computes heads {2c%16, 2c%16+1} for BOTH
    batches (2 head-instances x 2 batches), all 1024 tokens each.
  - AllToAll(o) [0.5MB bf16, 8-core mesh] redistributes attention outputs
    from head-sharded to token-sharded; reads are rank-agnostic because
    output block s holds sender s's 128 feature rows for MY tokens.
  - Out-projection with FULL Wo, MLP with FULL W1/W2: local per token
    shard, zero collectives (weights streamed from HBM, prefetched).
Final LN local; AllGather(hf); logits = [2048 tokens x 4000 vocab shard]
per core, written bf16 and converted to f32 on the host.
"""

import sys
import types

import numpy as np


def _install_ntff_shim():
    """Register the NTFF profiling hook that trn_boot skipped (the image's
    antenv package lacks the axon_hooks submodule)."""
    if "antenv.axon_hooks" in sys.modules:
        return
    try:
        import trn_agent_boot.trn_boot as tb
        hook = tb._ntff_profile_via_ctypes("/opt/axon/libaxon_pjrt.so")
    except Exception:
        hook = None
    mod = types.ModuleType("antenv.axon_hooks")
    _h = [hook]
    mod.get_axon_ntff_profile_hook = lambda: _h[0]
    mod.set_axon_ntff_profile_hook = lambda h: _h.__setitem__(0, h)
    sys.modules["antenv.axon_hooks"] = mod
    try:
        import antenv
        antenv.axon_hooks = mod
    except Exception:
        pass


_install_ntff_shim()

import ml_dtypes
import concourse.bass as bass
import concourse.mybir as mybir
import concourse.tile as tile
from concourse import bacc
from concourse.bass_utils import run_bass_kernel_spmd

BF = mybir.dt.bfloat16
F32 = mybir.dt.float32
AL = mybir.AluOpType
AF = mybir.ActivationFunctionType

CFG = dict(B=2, S=1024, V=32000, D=1024, H=16, L=4, EPS=1e-5)

N_CORES = 8


def build_program(cfg=None):
    """Build the SPMD Bass program (identical on all 8 cores)."""
    c = dict(CFG)
    if cfg:
        c.update(cfg)
    B, S, V, D, H, L = c["B"], c["S"], c["V"], c["D"], c["H"], c["L"]
    EPS = c["EPS"]
    T = S                    # tokens per batch (1024)
    GT = B * S               # global tokens (2048)
    TL = GT // N_CORES       # tokens per core (256)
    DK = D // H              # head dim (64)
    HL = H // N_CORES * B    # head-instances per core... (2 heads)
    DF = 4 * D
    VSH = V // N_CORES       # vocab shard (4000)
    KT = D // 128            # feature k-tiles (8)
    KTF = DF // 128          # mlp hidden k-tiles (32)
    NCH = T // 512           # attention token chunks per batch (2)
    TCH = 512                # attention token chunk size
    TT = T // 128            # token tiles per batch (8)
    GTT = GT // 128          # global token tiles (16)
    VCH = 500                # vocab chunk
    NV = VSH // VCH          # vocab n-chunks (8)
    NE = DF // 512           # mlp hidden eighth-chunks (8)
    HLC = 2                  # heads per core

    groups = [list(range(N_CORES))]

    nc = bacc.Bacc("TRN2", target_bir_lowering=False, debug=False,
                   num_devices=N_CORES)

    # ---- DRAM parameters (per-core shards fed via in_maps) ----
    xT0 = nc.dram_tensor("xT0", [D, TL], F32, kind="ExternalInput")
    wq = nc.dram_tensor("wq", [L, D, HLC * DK], BF, kind="ExternalInput")
    wk = nc.dram_tensor("wk", [L, D, HLC * DK], BF, kind="ExternalInput")
    wv = nc.dram_tensor("wv", [L, D, HLC * DK], BF, kind="ExternalInput")
    wo = nc.dram_tensor("wo", [L, D, D], BF, kind="ExternalInput")
    w1 = nc.dram_tensor("w1", [L, D, DF], BF, kind="ExternalInput")
    w2 = nc.dram_tensor("w2", [L, DF, D], BF, kind="ExternalInput")
    b1 = nc.dram_tensor("b1", [L, DF], F32, kind="ExternalInput")
    b2 = nc.dram_tensor("b2", [L, D], F32, kind="ExternalInput")
    g1 = nc.dram_tensor("g1", [L, D], F32, kind="ExternalInput")
    be1 = nc.dram_tensor("be1", [L, D], F32, kind="ExternalInput")
    g2 = nc.dram_tensor("g2", [L, D], F32, kind="ExternalInput")
    be2 = nc.dram_tensor("be2", [L, D], F32, kind="ExternalInput")
    gf = nc.dram_tensor("gf", [1, D], F32, kind="ExternalInput")
    bef = nc.dram_tensor("bef", [1, D], F32, kind="ExternalInput")
    hw = nc.dram_tensor("hw", [D, VSH], BF, kind="ExternalInput")
    logits = nc.dram_tensor("logits", [GT, VSH], BF, kind="ExternalOutput")

    with tile.TileContext(nc) as tc:
        _build_tc(nc, tc, locals())
    nc.compile()
    return nc


def _build_tc(nc, tc, v):
    """Emit the tile program. `v` is the name->value dict from build_program."""
    (B, T, GT, TL, D, L, EPS, DK, DF, KT, KTF, NCH, TCH, TT, GTT, VCH, NV,
     NE, HLC, groups) = (
        v["B"], v["T"], v["GT"], v["TL"], v["D"], v["L"], v["EPS"], v["DK"],
        v["DF"], v["KT"], v["KTF"], v["NCH"], v["TCH"], v["TT"], v["GTT"],
        v["VCH"], v["NV"], v["NE"], v["HLC"], v["groups"])
    xT0, wq, wk, wv, wo, w1, w2 = (v["xT0"], v["wq"], v["wk"], v["wv"],
                                   v["wo"], v["w1"], v["w2"])
    b1d, b2d, g1d, be1d, g2d, be2d, gfd, befd = (
        v["b1"], v["b2"], v["g1"], v["be1"], v["g2"], v["be2"], v["gf"],
        v["bef"])
    hwd, logits = v["hw"], v["logits"]
    NC = N_CORES

    import contextlib
    ctx = contextlib.ExitStack()

    # ---------------- pools ----------------
    sing = ctx.enter_context(tc.tile_pool(name="sing", bufs=1))
    wts = ctx.enter_context(tc.tile_pool(name="wts", bufs=1))
    wos = ctx.enter_context(tc.tile_pool(name="wos", bufs=1))
    w1s = ctx.enter_context(tc.tile_pool(name="w1s", bufs=2))
    w2s = ctx.enter_context(tc.tile_pool(name="w2s", bufs=8))
    hwp = ctx.enter_context(tc.tile_pool(name="hwp", bufs=2))
    hloc = ctx.enter_context(tc.tile_pool(name="hloc", bufs=2))
    hful = ctx.enter_context(tc.tile_pool(name="hful", bufs=1))
    qkp = ctx.enter_context(tc.tile_pool(name="qkp", bufs=1))
    scr = ctx.enter_context(tc.tile_pool(name="scr", bufs=2))
    expp = ctx.enter_context(tc.tile_pool(name="expp", bufs=16))
    otp = ctx.enter_context(tc.tile_pool(name="otp", bufs=1))
    up = ctx.enter_context(tc.tile_pool(name="up", bufs=1))
    lgp = ctx.enter_context(tc.tile_pool(name="lgp", bufs=2))
    tiny = ctx.enter_context(tc.tile_pool(name="tiny", bufs=2))
    rows1 = ctx.enter_context(tc.tile_pool(name="rows1", bufs=1))
    rows3 = ctx.enter_context(tc.tile_pool(name="rows3", bufs=3))
    rows2 = ctx.enter_context(tc.tile_pool(name="rows2", bufs=1))
    bcp = ctx.enter_context(tc.tile_pool(name="bcp", bufs=1))
    rbp = ctx.enter_context(tc.tile_pool(name="rbp", bufs=2))
    psmm = ctx.enter_context(tc.tile_pool(name="psmm", bufs=4, space="PSUM"))
    psz = ctx.enter_context(tc.tile_pool(name="psz", bufs=4, space="PSUM"))
    dram = ctx.enter_context(tc.tile_pool(name="dram", bufs=1, space="DRAM"))

    # ---------------- constants ----------------
    ones_col = sing.tile([128, 1], BF, name="ones_col")
    nc.vector.memset(ones_col, 1.0)
    eps_ap = sing.tile([1, 1], F32, name="eps_ap")
    nc.vector.memset(eps_ap, EPS)

    # ---------------- residual stream: x [128, KT, TL] fp32 ----------------
    x = sing.tile([128, KT, TL], F32, name="x")
    for k in range(KT):
        nc.sync.dma_start(out=x[:, k, :], in_=xT0[k * 128:(k + 1) * 128, :])

    # ---------------- layernorm over local tokens ----------------
    def layernorm(grow_dram, brow_dram, grow2, brow2, name, hview):
        """LN over the feature (partition) axis of transposed activations.
        x: [128, KT, TL] fp32.  Writes bf16 LN output into hview[:, k, :]."""
        gcol = tiny.tile([128, KT], F32, name=f"g_{name}", tag="gcol")
        nc.sync.dma_start(out=gcol, in_=grow_dram.rearrange("(k p) -> p k", p=128))
        gb = tiny.tile([33, D], F32, name=f"gb_{name}", tag="gbrow")
        nc.sync.dma_start(out=gb[0:1, :], in_=grow2)
        nc.sync.dma_start(out=gb[32:33, :], in_=brow2)

        # stats: ONE matmul per k-tile over [x | x^2]; PSUM row = [sum|sumsq]
        ps_st = psmm.tile([1, 2 * TL], F32, name="ps_st", tag="mm")
        for k in range(KT):
            xsq = scr.tile([128, 2 * TL], BF, name="xsq", tag="xsq")
            nc.vector.tensor_copy(xsq[:, 0:TL], x[:, k, :])
            nc.scalar.square(xsq[:, TL:2 * TL], xsq[:, 0:TL])
            nc.tensor.matmul(ps_st, ones_col, xsq,
                             start=(k == 0), stop=(k == KT - 1))
        mom = rows1.tile([1, 2 * TL], F32, name=f"mom_{name}", tag="mom")
        nc.scalar.mul(mom, ps_st, 1.0 / D)      # [mean | E[x^2]]
        mean = mom[:, 0:TL]
        msq = mom[:, TL:2 * TL]
        m2 = rows3.tile([1, TL], F32, name=f"m2_{name}", tag="row1k")
        nc.vector.tensor_mul(m2, mean, mean)
        var = rows3.tile([1, TL], F32, name=f"var_{name}", tag="row1k")
        nc.vector.tensor_tensor(out=var, in0=msq, in1=m2, op=AL.subtract)
        sd = rows3.tile([1, TL], F32, name=f"sd_{name}", tag="row1k")
        nc.scalar.activation(sd, var, AF.Sqrt, bias=eps_ap)
        rstd = rows3.tile([1, TL], F32, name=f"rstd_{name}", tag="row1k")
        nc.vector.reciprocal(rstd, sd)
        # er rows: [0]=e=-mean*rstd, [32]=1, rest 0 (engine partition
        # starts must be 32-aligned; zero rows nullify gb's garbage rows)
        er = rows3.tile([33, TL], F32, name=f"er_{name}", tag="er2")
        nc.vector.memset(er, 0.0)
        nc.vector.memset(er[32:33, :], 1.0)
        nc.vector.scalar_tensor_tensor(out=er[0:1, :], in0=mean, scalar=-1.0,
                                       in1=rstd, op0=AL.mult, op1=AL.mult)
        rstdB = bcp.tile([128, TL], F32, name="rstdB", tag="rstdB")
        nc.gpsimd.partition_broadcast(rstdB, rstd)
        # apply: h = (x*g)*rstdB + (g*e + b) ; affine term stays in PSUM
        for k in range(KT):
            nm4 = psmm.tile([128, TL], F32, name="nm4", tag="mm")
            nc.tensor.matmul(nm4, gb[:, k * 128:(k + 1) * 128], er,
                             start=True, stop=True)
            t1 = scr.tile([128, TL], F32, name="lnt", tag="lnt")
            nc.vector.scalar_tensor_tensor(
                out=t1, in0=x[:, k, :], scalar=gcol[:, k:k + 1],
                op0=AL.mult, in1=rstdB, op1=AL.mult)
            nc.vector.tensor_tensor(out=hview[:, k, :], in0=t1, in1=nm4,
                                    op=AL.add)

    def allgather_h(hloc_tile, tag_prefix, lname):
        """AllGather a local [128, KT, TL] bf16 tile -> full [128, KT, GT]."""
        h_in = dram.tile([128, KT * TL], BF, name=f"{tag_prefix}_in{lname}",
                         tag="h_in")
        h_out = dram.tile([NC * 128, KT * TL], BF,
                          name=f"{tag_prefix}_out{lname}", tag="h_out",
                          addr_space="Shared")
        nc.sync.dma_start(
            out=h_in.rearrange("p (k c) -> p k c", k=KT), in_=hloc_tile)
        nc.gpsimd.collective_compute(
            "AllGather", AL.bypass, replica_groups=groups,
            ins=[h_in.opt()], outs=[h_out.opt()])
        hf = hful.tile([128, KT, GT], BF, name=f"hf_{tag_prefix}", tag="hf")
        hov = h_out.rearrange("(s p) (k c) -> k p s c", p=128, c=TL)
        for k in range(KT):
            nc.sync.dma_start(
                out=hf[:, k, :].rearrange("p (s c) -> p s c", s=NC),
                in_=hov[k])
        return hf

    # ---------------- transformer layers ----------------
    for l in range(L):
        # -- LN1 (local tokens) + AllGather h1; weight loads issued after
        #    the AG trigger so they don't queue ahead of the h1 path --
        h1r = hloc.tile([128, KT, TL], BF, name="h1r", tag="hr")
        layernorm(g1d[l], be1d[l], g1d[l:l + 1, :], be1d[l:l + 1, :], f"ln1_{l}", h1r)
        h1f = allgather_h(h1r, "h1", str(l))
        wqt = wts.tile([128, KT, 128], BF, name="wqt", tag="wqt")
        wkt = wts.tile([128, KT, 128], BF, name="wkt", tag="wkt")
        wvt = wts.tile([128, KT, 128], BF, name="wvt", tag="wvt")
        for dst, src in ((wqt, wq), (wkt, wk), (wvt, wv)):
            nc.sync.dma_start(
                out=dst, in_=src[l].rearrange("(k p) m -> p k m", p=128))
        wot = wos.tile([128, KT, D], BF, name="wot", tag="wot")
        nc.sync.dma_start(out=wot, in_=wo[l].rearrange("(k p) m -> p k m", p=128))
        b1col = tiny.tile([128, KTF], F32, name="b1col", tag="b1col")
        nc.sync.dma_start(out=b1col, in_=b1d[l].rearrange("(k p) -> p k", p=128))
        b2col = tiny.tile([128, KT], F32, name="b2col", tag="b2col")
        nc.sync.dma_start(out=b2col, in_=b2d[l].rearrange("(k p) -> p k", p=128))

        # -- Q/K projections (transposed [128, B, T]) --
        qT = qkp.tile([128, B, T], BF, name="qT", tag="qT")
        kTt = qkp.tile([128, B, T], BF, name="kT", tag="kT")
        for wt, dst in ((wqt, qT), (wkt, kTt)):
            pq = {}
            for b in range(B):
                for chn in range(NCH):
                    pq[(b, chn)] = psmm.tile([128, TCH], F32, name="ps",
                                             tag="mm")
            for k in range(KT):
                for b in range(B):
                    for chn in range(NCH):
                        gs = slice(b * T + chn * TCH, b * T + (chn + 1) * TCH)
                        nc.tensor.matmul(pq[(b, chn)],
                                         wt[:, k, :],
                                         h1f[:, k, gs],
                                         start=(k == 0), stop=(k == KT - 1))
            for b in range(B):
                for chn in range(NCH):
                    cs = slice(chn * TCH, (chn + 1) * TCH)
                    nc.vector.tensor_copy(dst[:, b, cs], pq[(b, chn)])
        # -- V in natural layout [token, head, dk+1], per batch --
        vt = qkp.tile([128, B * TT, HLC, DK + 1], BF, name="vt", tag="vt")
        nc.vector.memset(vt[:, :, :, DK:DK + 1], 1.0)
        for b in range(B):
            for t in range(TT):
                ps = psmm.tile([128, TCH], F32, name="psv", tag="mm")
                for k in range(KT):
                    nc.tensor.matmul(
                        ps[:, 0:128],
                        h1f[:, k, b * T + t * 128:b * T + (t + 1) * 128],
                        wvt[:, k, :],
                        start=(k == 0), stop=(k == KT - 1))
                nc.vector.tensor_copy(
                    vt[:, b * TT + t, :, 0:DK],
                    ps[:, 0:128].rearrange("p (h d) -> p h d", h=HLC))

        # -- attention: both local heads interleaved (concurrent PE row
        #    groups at base partitions 0/64), all T tokens per batch --
        oT = otp.tile([128, B, T], BF, name="oT", tag="oT")
        for b in range(B):
            for chn in range(NCH):
                cs = slice(chn * TCH, (chn + 1) * TCH)
                jmax = (chn + 1) * (TCH // 128)
                exps = {}
                for j in range(jmax):
                    for hh in range(HLC):
                        po = hh * DK
                        pss = psmm.tile([128, TCH], F32, name="pss", tag="mm")
                        nc.tensor.matmul(
                            pss, kTt[po:po + DK, b, j * 128:(j + 1) * 128],
                            qT[po:po + DK, b, cs], start=True, stop=True)
                        et = expp.tile([128, TCH], BF, name="exp", tag="exp")
                        nc.scalar.activation(et, pss, AF.Exp, scale=0.125)
                        if j * 128 >= chn * TCH:
                            # diagonal block: zero where tk_global > tq_global
                            nc.gpsimd.affine_select(
                                out=et, in_=et, pattern=[[1, TCH]],
                                compare_op=AL.is_ge, fill=0.0,
                                base=chn * TCH - j * 128,
                                channel_multiplier=-1)
                        exps[(j, hh)] = et
                for hh in range(HLC):
                    po = hh * DK
                    ps_o = psz.tile([DK + 1, TCH], F32, name="ps_o",
                                    tag="zmm")
                    for j in range(jmax):
                        nc.tensor.matmul(ps_o, vt[:, b * TT + j, hh, :],
                                         exps[(j, hh)],
                                         start=(j == 0), stop=(j == jmax - 1))
                    rec = rows2.tile([1, TCH], F32, name="rec", tag="rec")
                    den = rows2.tile([1, TCH], F32, name="den", tag="den")
                    nc.vector.tensor_copy(den, ps_o[DK:DK + 1, :])
                    rsc = rows2.tile([1, TCH], F32, name="rsc", tag="rsc")
                    nc.vector.reciprocal_approx_accurate(rec, den, rsc)
                    recb = rows2.tile([1, TCH], BF, name="recb", tag="recb")
                    nc.vector.tensor_copy(recb, rec)
                    rb = rbp.tile([DK, TCH], BF, name="rb", tag="rb")
                    nc.gpsimd.partition_broadcast(rb, recb)
                    nc.vector.tensor_tensor(
                        out=oT[po:po + DK, b, cs], in0=ps_o[0:DK, :],
                        in1=rb, op=AL.mult)

        # -- AllToAll: head-sharded o -> token-sharded o_full --
        # in block d (global token block) = o[:, d's 256 tokens];
        # out block s = sender s's 128 feature rows for MY tokens.
        o_in = dram.tile([NC, 128 * TL], BF, name=f"o_in{l}", tag="o_in")
        o_out = dram.tile([NC, 128 * TL], BF, name=f"o_out{l}", tag="o_out")
        oiv = o_in.rearrange("d (p c) -> d p c", p=128)
        for d in range(NC):
            b, jj = d // 4, d % 4
            nc.sync.dma_start(out=oiv[d],
                              in_=oT[:, b, jj * TL:(jj + 1) * TL])
        nc.gpsimd.collective_compute(
            "AllToAll", AL.bypass, replica_groups=groups,
            ins=[o_in.opt()], outs=[o_out.opt()])
        oov = o_out.rearrange("s (p c) -> s p c", p=128)
        o_tok = otp.tile([128, KT, TL], BF, name="o_tok", tag="o_tok")
        for k in range(KT):
            nc.sync.dma_start(out=o_tok[:, k, :], in_=oov[k])

        # -- attn out projection (local tokens, full Wo): d1 = Wo^T o_tok --
        # one PSUM bank per output m-tile (a bank may hold only ONE
        # accumulation region: start=True clears has_written for the whole
        # bank on the written partitions)
        for m in range(KT):
            d1_ps = psmm.tile([128, TCH], F32, name="d1ps", tag="mm")
            for k in range(KT):
                nc.tensor.matmul(
                    d1_ps[:, 0:TL],
                    wot[:, k, m * 128:(m + 1) * 128],
                    o_tok[:, k, :],
                    start=(k == 0), stop=(k == KT - 1))
            nc.vector.tensor_tensor(
                out=x[:, m, :], in0=d1_ps[:, 0:TL],
                in1=x[:, m, :], op=AL.add)

        # -- LN2 + MLP (token-sharded, full W1/W2) --
        h2r = hloc.tile([128, KT, TL], BF, name="h2r", tag="hr")
        layernorm(g2d[l], be2d[l], g2d[l:l + 1, :], be2d[l:l + 1, :], f"ln2_{l}", h2r)
        ut = up.tile([128, KTF, TL], BF, name="ut", tag="ut")
        for e in range(NE):
            w1e = w1s.tile([128, KT, 512], BF, name="w1e", tag="w1e")
            nc.sync.dma_start(
                out=w1e,
                in_=w1[l][:, e * 512:(e + 1) * 512].rearrange(
                    "(k p) m -> p k m", p=128))
            for mm in range(4):
                kf = e * 4 + mm
                pu = psmm.tile([128, TCH], F32, name="psu", tag="mm")
                for k in range(KT):
                    nc.tensor.matmul(
                        pu[:, 0:TL],
                        w1e[:, k, mm * 128:(mm + 1) * 128],
                        h2r[:, k, :],
                        start=(k == 0), stop=(k == KT - 1))
                nc.scalar.activation(
                    ut[:, kf, :], pu[:, 0:TL],
                    AF.Gelu, bias=b1col[:, kf:kf + 1])
        # z = W2^T u accumulated in 4 k-groups of 8; each group's partial
        # sums drain straight into x (b2 folded into the first group)
        KG = 4
        for kg in range(KG):
            w2g = []
            for kk in range(KTF // KG):
                k = kg * (KTF // KG) + kk
                w2k = w2s.tile([128, D], BF, name="w2k", tag="w2k")
                nc.sync.dma_start(out=w2k,
                                  in_=w2[l][k * 128:(k + 1) * 128, :])
                w2g.append(w2k)
            for m in range(KT):
                z_ps = psz.tile([128, TCH], F32, name="zps", tag="zmm")
                for kk in range(KTF // KG):
                    k = kg * (KTF // KG) + kk
                    nc.tensor.matmul(
                        z_ps[:, 0:TL],
                        w2g[kk][:, m * 128:(m + 1) * 128],
                        ut[:, k, :],
                        start=(kk == 0), stop=(kk == KTF // KG - 1))
                if kg == 0:
                    # x = (z_part + b2) + x
                    nc.vector.scalar_tensor_tensor(
                        out=x[:, m, :], in0=z_ps[:, 0:TL],
                        scalar=b2col[:, m:m + 1], in1=x[:, m, :],
                        op0=AL.add, op1=AL.add)
                else:
                    nc.vector.tensor_tensor(
                        out=x[:, m, :], in0=z_ps[:, 0:TL],
                        in1=x[:, m, :], op=AL.add)

    # ---------------- final LN + AllGather + logits ----------------
    hfr = hloc.tile([128, KT, TL], BF, name="hfr", tag="hr")
    layernorm(gfd[0], befd[0], gfd[0:1, :], befd[0:1, :], "lnf", hfr)
    hff = allgather_h(hfr, "hf", "")
    for n in range(NV):
        hwb = hwp.tile([128, KT, VCH], BF, name="hwb", tag="hwb")
        nc.sync.dma_start(
            out=hwb,
            in_=hwd[:, n * VCH:(n + 1) * VCH].rearrange(
                "(k p) m -> p k m", p=128))
        for t in range(GTT):
            ps = psmm.tile([128, TCH], F32, name="pslg", tag="mm")
            for k in range(KT):
                nc.tensor.matmul(ps[:, 0:VCH],
                                 hff[:, k, t * 128:(t + 1) * 128],
                                 hwb[:, k, :],
                                 start=(k == 0), stop=(k == KT - 1))
            lg = lgp.tile([128, VCH], BF, name="lg", tag="lg")
            nc.vector.tensor_copy(lg, ps[:, 0:VCH])
            nc.sync.dma_start(
                out=logits[t * 128:(t + 1) * 128, n * VCH:(n + 1) * VCH],
                in_=lg)

    ctx.close()


# ---------------- host side ----------------

_PROG_CACHE = {}


def _get_program():
    if "nc" not in _PROG_CACHE:
        _PROG_CACHE["nc"] = build_program()
    return _PROG_CACHE["nc"]


def make_in_maps(input_ids, emb, Wq, Wk, Wv, Wo, W1, b1, W2, b2,
                 ln1_g, ln1_b, ln2_g, ln2_b, lnf_g, lnf_b, head_w):
    D, V, S, B = CFG["D"], CFG["V"], CFG["S"], CFG["B"]
    TL = B * S // N_CORES
    VSH = V // N_CORES
    bf = ml_dtypes.bfloat16
    # shared full tensors (cast once)
    wo_f = np.ascontiguousarray(Wo).astype(bf)
    w1_f = np.ascontiguousarray(W1).astype(bf)
    w2_f = np.ascontiguousarray(W2).astype(bf)
    b1_f = np.asarray(b1, dtype=np.float32)
    b2_f = np.asarray(b2, dtype=np.float32)
    g1_f = np.asarray(ln1_g, dtype=np.float32)
    be1_f = np.asarray(ln1_b, dtype=np.float32)
    g2_f = np.asarray(ln2_g, dtype=np.float32)
    be2_f = np.asarray(ln2_b, dtype=np.float32)
    gf_f = np.asarray(lnf_g, dtype=np.float32).reshape(1, -1)
    bef_f = np.asarray(lnf_b, dtype=np.float32).reshape(1, -1)
    # per-core head shards (2 heads = 128 cols) and vocab shards
    wq_sh = [np.ascontiguousarray(Wq[:, :, c * 128:(c + 1) * 128]).astype(bf)
             for c in range(N_CORES)]
    wk_sh = [np.ascontiguousarray(Wk[:, :, c * 128:(c + 1) * 128]).astype(bf)
             for c in range(N_CORES)]
    wv_sh = [np.ascontiguousarray(Wv[:, :, c * 128:(c + 1) * 128]).astype(bf)
             for c in range(N_CORES)]
    hw_sh = [np.ascontiguousarray(
        head_w[:, c * VSH:(c + 1) * VSH]).astype(bf) for c in range(N_CORES)]
    x0T = []
    for g in range(B):
        x0 = np.asarray(emb)[np.asarray(input_ids)[g]]          # [S, D] f32
        x0T.append(np.ascontiguousarray(x0.T).astype(np.float32))
    in_maps = []
    for c in range(N_CORES):
        g, r = c // 4, c % 4
        in_maps.append({
            "xT0": np.ascontiguousarray(x0T[g][:, r * 256:(r + 1) * 256]),
            "wq": wq_sh[c],
            "wk": wk_sh[c],
            "wv": wv_sh[c],
            "wo": wo_f,
            "w1": w1_f,
            "w2": w2_f,
            "b1": b1_f,
            "b2": b2_f,
            "g1": g1_f,
            "be1": be1_f,
            "g2": g2_f,
            "be2": be2_f,
            "gf": gf_f,
            "bef": bef_f,
            "hw": hw_sh[c],
        })
    return in_maps


def _assemble(res):
    B, S, V = CFG["B"], CFG["S"], CFG["V"]
    VSH = V // N_CORES
    out = np.empty((B, S, V), dtype=np.float32)
    for c in range(N_CORES):
        lg = np.asarray(res.results[c]["logits"], dtype=np.float32)
        for g in range(B):
            out[g, :, c * VSH:(c + 1) * VSH] = lg[g * S:(g + 1) * S, :]
    return out


def kernel(**inputs):
    nc = _get_program()
    in_maps = make_in_maps(**inputs)
    res = run_bass_kernel_spmd(nc, in_maps, list(range(N_CORES)), trace=False)
    return _assemble(res)


def run_traced(**inputs):
    """Like kernel() but with NTFF tracing; returns (out, exec_time_ns)."""
    nc = _get_program()
    in_maps = make_in_maps(**inputs)
    res = run_bass_kernel_spmd(nc, in_maps, list(range(N_CORES)), trace=True)
    return _assemble(res), res.exec_time_ns
